# revision 9
# baseline (speedup 1.0000x reference)
"""DBLoss (OHEM-masked BCE + masked L1 threshold loss) on 8 Trainium2 cores.

Shapes are hardcoded for the nn_DBLoss problem:
  outputs             [16, 3, 640, 640] f32
  gt_shrink_labels    [16, 640, 640]    f32
  gt_threshold_labels [16, 640, 640]    f32
Returns np.float32[4] = (loss_all, loss_shrink, loss_binary, loss_thresh).

Sharding: pure data parallel — 2 images per core, 8 cores. Each core computes
per-image partial sums (per-partition vectors); the host reduces the tiny
partials and forms the masked means.

Math notes (device fast path):
 * OHEM: with neg_num == neg_total (i.e. 3*pos_num >= neg_total) the top-k
   threshold is the minimum negative score, so the selection mask is exactly
   all-ones for every valid image. The host verifies this condition per image
   (along with pos_num>0, neg_total>0) and falls back to an exact numpy
   implementation if any image needs a true top-k (cannot happen for the
   problem's uniform-random labels).
 * BCE with binarized target t and no sigmoid clipping reduces to
   softplus(x) - t*x; the host verifies |logits| < 16 so the 1e-7 clip in the
   reference is inactive.
 * threshold-loss L1 per image: sum|u - gtt| = 2*sum max(u, gtt) - sum u
   - sum gtt, with u = sigmoid(tm).  sum max accumulates on DVE, sum u rides
   free on the ACT sigmoid op's accum_out, and sum gtt is summed from the
   input on the host in float64.
 * threshold-loss mask (gt_t>0)|(gt_s>0): the device sums over all pixels;
   the host subtracts exact corrections for the (measure-zero) pixels where
   both labels are <= 0.

DMA layout (v6): SDMA engine 15 — which serves SBUF partitions {92-95,
124-127} — is measurably ~20% slower than the other fifteen engines on this
part. With standard full-partition descriptors every plane's completion
semaphore is gated by engine 15, so the whole pipeline paces at its rate
(~344 GB/s effective instead of ~430). v6 therefore loads each plane as two
partition-range descriptors [0:92] and [96:124], which the partition->port
map spreads evenly over engines 0-14 (8 rows each) and engine 15 not at
all. Partitions 92-95/124-127 are never written: compute ops still run the
full [128, 3200] shape, and whatever garbage sits in those partitions stays
isolated in their per-partition accumulator rows, which the host drops and
replaces with exact float64 corrections computed from the inputs (8 rows x
3200 px per plane).

Schedule: one HWDGE ring (sync engine), plane order tm0 tm1 s0 g0 bn0 s1 g1
bn1 gtt1 gtt0 (the gtt planes last: their only consumer is one DVE
max-accum per col-half with u ready long before, so trailing compute is
~2us). Per-plane semaphores with threshold = 16 * (number of descriptors)
make the waits sound regardless of how the DGE splits a descriptor across
engines (engines with no rows post their increment at issue time, so
partial thresholds on a shared semaphore would fire early). ACT switches
activation tables exactly once (sigmoid set -> exp/ln set); both table
loads are pulled early / hidden by dummy 1-element activations.
"""

import sys

import numpy as np

try:
    import concourse.bass as bass
except ImportError:  # stand-alone grading dir: fall back to known repo paths
    for _p in ("/root/.axon_site/_ro/trn_rl_repo", "/opt/trn_rl_repo"):
        if _p not in sys.path:
            sys.path.append(_p)
    import concourse.bass as bass

from concourse import mybir
from concourse.bass_utils import run_bass_kernel_spmd

B, H, W = 16, 640, 640
N = H * W                    # 409600 pixels / image
P = 128                      # SBUF partitions (92:96 and 124:128 unloaded)
F = N // P                   # 3200 free elements / partition
NCORES = 8
BPC = B // NCORES            # 2 images per core
ALPHA, BETA = 1.0, 10.0
F32 = mybir.dt.float32

# partition ranges loaded by DMA (avoid SDMA engine 15's partitions)
PA, PB = (0, 92), (96, 124)
# flat element indices of the host-corrected rows, per plane
_R_IDX = np.r_[92 * F:96 * F, 124 * F:128 * F]
_CLEAN_ROWS = np.r_[0:92, 96:124]

_CACHED_NC = None

# po column layout ([128, 14] partial sums; host uses rows 0:92+96:124):
#  0 sum softplus(s0)   1 sum softplus(bn0)
#  2 sum softplus(s1)   3 sum softplus(bn1)
#  4 sum t0*s0   5 sum t0*bn0   6 sum t1*s1   7 sum t1*bn1
#  8 sum sigmoid(tm0)   9 sum sigmoid(tm1)
#  10/11 sum max(u1,gtt1) col-halves   12/13 sum max(u0,gtt0) col-halves
PCOLS = 14


def build_nc() -> "bass.Bass":
    """Per-core raw-bass program (see module docstring for the schedule).

    Raw bass (no TileContext): this walrus build encodes at most ONE attached
    sync-wait per TPB instruction, so cross-engine data deps use standalone
    wait_ge instructions; every ACT/DVE op carries an attached wait on the
    previous same-engine op's write-ack (acks are cumulative), which covers
    all same-engine RAW/WAW at ~30ns cost.

    Semaphores: one per plane (threshold 32 = both partition-range
    descriptors complete), except the gtt planes which get one per
    descriptor group (m2 = [96:124] full width; a/b = [0:92] col-halves) so
    the max-accum per col-half can start as soon as its half landed.
    sa/sv = ACT/DVE op counters, sc = bias-constant memset done, dout =
    output DMA done. Sync clears every semaphore at the end so repeated
    executions of the loaded NEFF start from zero.
    """
    nc = bass.Bass(dynamic_dma_scratch_size=2048, enable_partition_id=False,
                   monotonic_sem_count=0)
    outs = nc.dram_tensor("outs", [BPC, 3, N], F32, kind="ExternalInput")
    gts = nc.dram_tensor("gts", [BPC, N], F32, kind="ExternalInput")
    gtt = nc.dram_tensor("gtt", [BPC, N], F32, kind="ExternalInput")
    part = nc.dram_tensor("part", [P, PCOLS], F32, kind="ExternalOutput")

    ag = mybir.AluOpType.is_gt
    mul = mybir.AluOpType.mult
    add = mybir.AluOpType.add
    amax = mybir.AluOpType.max
    fexp = mybir.ActivationFunctionType.Exp
    fln = mybir.ActivationFunctionType.Ln
    fsig = mybir.ActivationFunctionType.Sigmoid
    h = F // 2

    from contextlib import ExitStack
    ctx = ExitStack()
    with ctx:
        sb = lambda nm, shape: ctx.enter_context(nc.sbuf_tensor(nm, shape, F32))
        sem = lambda nm: ctx.enter_context(nc.semaphore(name=nm))
        tm = [sb("tm_0", [P, F]), sb("tm_1", [P, F])]
        s = [sb("s_0", [P, F]), sb("s_1", [P, F])]
        bn = [sb("bn_0", [P, F]), sb("bn_1", [P, F])]
        g = [sb("g_0", [P, F]), sb("g_1", [P, F])]
        gt = [sb("gt_0", [P, F]), sb("gt_1", [P, F])]
        u = [sb("u_0", [P, F]), sb("u_1", [P, F])]
        eu = sb("eu", [P, F])
        tr = sb("tr", [P, F])
        po = sb("po", [P, PCOLS])
        bias1 = sb("bias1", [P, 1])
        dum = sb("dum", [P, 1])
        dtm0, dtm1, ds0, dg0, dbn0, ds1, dg1, dbn1 = (
            sem(nm) for nm in ("dtm0", "dtm1", "ds0", "dg0", "dbn0", "ds1",
                               "dg1", "dbn1"))
        dt1m2, dt1a, dt1b = sem("dt1m2"), sem("dt1a"), sem("dt1b")
        dt0m2, dt0a, dt0b = sem("dt0m2"), sem("dt0a"), sem("dt0b")
        sa, sv, sc, dout = sem("sa"), sem("sv"), sem("sc"), sem("dout")
        all_sems = [dtm0, dtm1, ds0, dg0, dbn0, ds1, dg1, dbn1,
                    dt1m2, dt1a, dt1b, dt0m2, dt0a, dt0b, sa, sv, sc, dout]
        block = ctx.enter_context(nc.Block(no_gpsimd_drain=True))

        pf = lambda t: t.rearrange("(p f) -> p f", p=P)

        @block.sync
        def _(sync):
            def load2(dst, src, dsem):
                # one plane as two partition-range descriptors; engines 0-14
                # get 8 rows each, engine 15 none. Sem thr 32 = both done.
                for lo, hi in (PA, PB):
                    sync.dma_start(out=dst[lo:hi, :], in_=src[lo:hi, :]
                                   ).then_inc(dsem, 16)

            load2(tm[0], pf(outs[0, 1]), dtm0)
            load2(tm[1], pf(outs[1, 1]), dtm1)
            load2(s[0], pf(outs[0, 0]), ds0)
            load2(g[0], pf(gts[0]), dg0)
            load2(bn[0], pf(outs[0, 2]), dbn0)
            load2(s[1], pf(outs[1, 0]), ds1)
            load2(g[1], pf(gts[1]), dg1)
            load2(bn[1], pf(outs[1, 2]), dbn1)
            for gt_t, src, (dm2, da_, db_) in (
                    (gt[1], pf(gtt[1]), (dt1m2, dt1a, dt1b)),
                    (gt[0], pf(gtt[0]), (dt0m2, dt0a, dt0b))):
                lo, hi = PB
                sync.dma_start(out=gt_t[lo:hi, :], in_=src[lo:hi, :]
                               ).then_inc(dm2, 16)
                lo, hi = PA
                sync.dma_start(out=gt_t[lo:hi, :h], in_=src[lo:hi, :h]
                               ).then_inc(da_, 16)
                sync.dma_start(out=gt_t[lo:hi, h:], in_=src[lo:hi, h:]
                               ).then_inc(db_, 16)
            sync.wait_ge(sa, 12)
            sync.wait_ge(sv, 8)
            sync.dma_start(out=part[:, :], in_=po[:, :]).then_inc(dout, 16)
            for semh in all_sems:
                if semh is not dout:
                    sync.sem_clear(semh)
            sync.wait_ge(dout, 16)
            sync.sem_clear(dout)

        @block.scalar
        def _(scalar):
            sa_n = 0

            def act(out, in_, func, **kw):
                nonlocal sa_n
                inst = nc.scalar.activation(out=out, in_=in_, func=func,
                                            **kw).then_inc(sa, 1)
                if sa_n >= 1:
                    inst.wait_op(sa, sa_n, "sem-ge")
                sa_n += 1

            # dummy 1-elem sigmoid: pulls the sigmoid table load early
            act(dum[:, :], dum[:, :], fsig)                       # sa 1
            scalar.wait_ge(dtm0, 32)
            act(u[0][:, :], tm[0][:, :], fsig,
                accum_out=po[:, 8:9])                             # sa 2
            scalar.wait_ge(dtm1, 32)
            act(u[1][:, :], tm[1][:, :], fsig,
                accum_out=po[:, 9:10])                            # sa 3
            # dummy 1-elem exp: triggers the exp/ln table switch now, so the
            # load hides under the s0 DMA instead of stalling behind it
            act(dum[:, :], dum[:, :], fexp)                       # sa 4
            scalar.wait_ge(sc, 1)
            plan = [  # (plane tile, plane sem, po col)
                (s[0], ds0, 0),
                (bn[0], dbn0, 1),
                (s[1], ds1, 2),
                (bn[1], dbn1, 3),
            ]
            for pl, dsem, col in plan:
                scalar.wait_ge(dsem, 32)
                act(eu[:, :], pl[:, :], fexp)
                act(eu[:, :], eu[:, :], fln, bias=bias1[:, :],
                    accum_out=po[:, col:col + 1])
            assert sa_n == 12

        @block.vector
        def _(vector):
            nc.vector.memset(bias1[:, :], 1.0).then_inc(sc, 1)
            sv_n = 0

            def stt(out, in0, scalar_v, in1, op0, op1, col):
                nonlocal sv_n
                inst = nc.vector.scalar_tensor_tensor(
                    out=out, in0=in0, scalar=scalar_v, in1=in1,
                    op0=op0, op1=op1, accum_out=po[:, col:col + 1],
                ).then_inc(sv, 1)
                if sv_n >= 1:
                    inst.wait_op(sv, sv_n, "sem-ge")
                sv_n += 1

            vector.wait_ge(dg0, 32)
            vector.wait_ge(ds0, 32)
            stt(tr[:, :], g[0][:, :], 0.5, s[0][:, :], ag, mul, 4)
            vector.wait_ge(dbn0, 32)
            stt(tr[:, :], g[0][:, :], 0.5, bn[0][:, :], ag, mul, 5)
            vector.wait_ge(dg1, 32)
            vector.wait_ge(ds1, 32)
            stt(tr[:, :], g[1][:, :], 0.5, s[1][:, :], ag, mul, 6)
            vector.wait_ge(dbn1, 32)
            stt(tr[:, :], g[1][:, :], 0.5, bn[1][:, :], ag, mul, 7)
            vector.wait_ge(sa, 3)
            vector.wait_ge(dt1m2, 16)
            vector.wait_ge(dt1a, 16)
            stt(tr[:, :h], u[1][:, :h], 0.0, gt[1][:, :h], add, amax, 10)
            vector.wait_ge(dt1b, 16)
            stt(tr[:, h:], u[1][:, h:], 0.0, gt[1][:, h:], add, amax, 11)
            vector.wait_ge(dt0m2, 16)
            vector.wait_ge(dt0a, 16)
            stt(tr[:, :h], u[0][:, :h], 0.0, gt[0][:, :h], add, amax, 12)
            vector.wait_ge(dt0b, 16)
            stt(tr[:, h:], u[0][:, h:], 0.0, gt[0][:, h:], add, amax, 13)
            assert sv_n == 8

    return nc


def _numpy_reference(outputs, gt_shrink_labels, gt_threshold_labels):
    """Exact fallback for inputs outside the fast-path regime."""
    OHEM_RATIO, EPS = 3, 1e-7

    def sigmoid(x):
        return 1.0 / (1.0 + np.exp(-x))

    shrink, thresh, binary = outputs[:, 0], outputs[:, 1], outputs[:, 2]
    b = outputs.shape[0]
    flat_s = shrink.reshape(b, -1)
    flat_pos = (gt_shrink_labels > 0.5).reshape(b, -1)
    n = flat_s.shape[1]
    pos_num = flat_pos.sum(axis=1)
    neg_total = n - pos_num
    neg_num = np.minimum(pos_num * OHEM_RATIO, neg_total)
    neg_scores = np.where(flat_pos, -np.inf, flat_s)
    sorted_desc = -np.sort(-neg_scores, axis=1)
    idx = np.clip(neg_num - 1, 0, n - 1).astype(np.int64)
    thr = np.take_along_axis(sorted_desc, idx[:, None], axis=1)
    mask = (flat_s >= thr) | flat_pos
    valid = (pos_num > 0) & (neg_num > 0)
    mask = (mask & valid[:, None]).reshape(shrink.shape).astype(np.float32)

    def masked_bce(logits, target, m):
        p = np.clip(sigmoid(logits), EPS, 1.0 - EPS)
        t = (target > 0.5).astype(np.float32)
        per_px = -(t * np.log(p) + (1.0 - t) * np.log(1.0 - p))
        denom = m.sum()
        return float(per_px.flatten() @ m.flatten() / max(denom, 1.0)) if denom > 0 else 0.0

    loss_shrink = masked_bce(shrink, gt_shrink_labels, mask)
    loss_binary = masked_bce(binary, gt_shrink_labels, mask)
    m2 = ((gt_threshold_labels > 0) | (gt_shrink_labels > 0)).astype(np.float32)
    denom2 = m2.sum()
    l1 = np.abs(sigmoid(thresh) - gt_threshold_labels).flatten() @ m2.flatten()
    loss_thresh = float(l1 / max(denom2, 1.0)) if denom2 > 0 else 0.0
    loss_all = loss_shrink + ALPHA * loss_binary + BETA * loss_thresh
    return np.array([loss_all, loss_shrink, loss_binary, loss_thresh], np.float32)


def kernel(outputs, gt_shrink_labels, gt_threshold_labels, _trace=False):
    global _CACHED_NC
    outputs = np.ascontiguousarray(np.asarray(outputs, dtype=np.float32))
    gts = np.ascontiguousarray(np.asarray(gt_shrink_labels, dtype=np.float32))
    gtt = np.ascontiguousarray(np.asarray(gt_threshold_labels, dtype=np.float32))

    # ---- host-side regime checks (exactness guards for the fast path) ----
    pos_num = (gts > 0.5).reshape(B, -1).sum(axis=1)
    neg_total = N - pos_num
    neg_num = np.minimum(3 * pos_num, neg_total)
    valid = (pos_num > 0) & (neg_num > 0)
    needs_topk = valid & (3 * pos_num < neg_total)
    clip_active = max(
        float(np.abs(outputs[:, 0]).max()), float(np.abs(outputs[:, 2]).max())
    ) >= 16.0
    if needs_topk.any() or clip_active:
        return _numpy_reference(outputs, gts, gtt)

    if _CACHED_NC is None:
        _CACHED_NC = build_nc()
    nc = _CACHED_NC

    in_maps = []
    for c in range(NCORES):
        sl = slice(c * BPC, (c + 1) * BPC)
        in_maps.append({
            "outs": outputs[sl].reshape(BPC, 3, N),
            "gts": gts[sl].reshape(BPC, N),
            "gtt": gtt[sl].reshape(BPC, N),
        })
    res = run_bass_kernel_spmd(
        nc, in_maps, core_ids=list(range(NCORES)), trace=_trace
    )

    # ---- host-side exact terms (float64): full gtt sums and the
    # corrections for the unloaded partitions' pixels (rows 92:96+124:128
    # of every plane) ----
    def softplus64(x):
        return np.logaddexp(0.0, x.astype(np.float64))

    def sigmoid64(x):
        return 1.0 / (1.0 + np.exp(-x.astype(np.float64)))

    of = outputs.reshape(B, 3, N)
    gsf = gts.reshape(B, N)
    gtf = gtt.reshape(B, N)
    sum_gtt = gtf.astype(np.float64).sum(axis=1)
    sR, tmR, bnR = of[:, 0, _R_IDX], of[:, 1, _R_IDX], of[:, 2, _R_IDX]
    gR, gttR = gsf[:, _R_IDX], gtf[:, _R_IDX]
    tmaskR = (gR > 0.5).astype(np.float64)
    uR = sigmoid64(tmR)
    sp_s_c = softplus64(sR).sum(axis=1)
    sp_b_c = softplus64(bnR).sum(axis=1)
    ts_c = (tmaskR * sR.astype(np.float64)).sum(axis=1)
    tb_c = (tmaskR * bnR.astype(np.float64)).sum(axis=1)
    u_c = uR.sum(axis=1)
    mx_c = np.maximum(uR, gttR.astype(np.float64)).sum(axis=1)

    # ---- host combine: per-image sums from per-partition partials ----
    sp_s = np.empty(B); sp_b = np.empty(B); ts = np.empty(B); tb = np.empty(B)
    l1 = np.empty(B)
    for c in range(NCORES):
        p = res.results[c]["part"][_CLEAN_ROWS].astype(np.float64).sum(axis=0)
        i0, i1 = c * BPC, c * BPC + 1
        sp_s[i0], sp_b[i0] = p[0] + sp_s_c[i0], p[1] + sp_b_c[i0]
        sp_s[i1], sp_b[i1] = p[2] + sp_s_c[i1], p[3] + sp_b_c[i1]
        ts[i0], tb[i0] = p[4] + ts_c[i0], p[5] + tb_c[i0]
        ts[i1], tb[i1] = p[6] + ts_c[i1], p[7] + tb_c[i1]
        l1[i0] = (2.0 * (p[12] + p[13] + mx_c[i0])
                  - (p[8] + u_c[i0]) - sum_gtt[i0])
        l1[i1] = (2.0 * (p[10] + p[11] + mx_c[i1])
                  - (p[9] + u_c[i1]) - sum_gtt[i1])

    cnt = float(N * valid.sum())
    num_s = float(((sp_s - ts) * valid).sum())
    num_b = float(((sp_b - tb) * valid).sum())
    loss_shrink = num_s / max(cnt, 1.0) if cnt > 0 else 0.0
    loss_binary = num_b / max(cnt, 1.0) if cnt > 0 else 0.0

    # threshold-loss mask corrections for pixels where both labels <= 0
    zz = (gtt <= 0) & (gts <= 0)
    cnt2 = float(B * N - zz.sum())
    l1_tot = float(l1.sum())
    if zz.any():
        tmz = outputs[:, 1][zz]
        l1_tot -= float(np.abs(1.0 / (1.0 + np.exp(-tmz)) - gtt[zz]).sum())
    loss_thresh = l1_tot / max(cnt2, 1.0) if cnt2 > 0 else 0.0

    loss_all = loss_shrink + ALPHA * loss_binary + BETA * loss_thresh
    out = np.array([loss_all, loss_shrink, loss_binary, loss_thresh], np.float32)
    if _trace:
        return out, res
    return out
